# revision 1
# baseline (speedup 1.0000x reference)
"""GCN (3-layer graph conv) Trainium2 kernel running SPMD on 8 NeuronCores.

Approach
--------
- Destination-node 1D sharding; nodes renumbered (degree-balanced snake
  deal across cores, degree-sorted within a core) and padded so each core
  owns R rows = NBLK blocks of 128.
- Per layer a bf16 feature table ([N_PAD, 128] in HBM, replicated per
  core via AllGather of per-core transform shards) is randomly gathered
  with gpsimd.dma_gather (int16 indices -> 4 source chunks of N_PAD/4
  rows each).
- SpMM: for each 128-row destination block, one-hot selection matrices
  (DVE tensor_scalar: is_equal(iota, row_local) * val) x gathered tiles
  (PE matmul) accumulate the block result in PSUM.
- The schedule is UNIFORM across cores (group counts per (block, chunk)
  padded to the max over cores) so one SPMD program serves all 8 cores;
  per-core behavior differs only through the idx/row-local/val streams.

kernel(**inputs) accepts the full-size inputs from reference.setup_inputs
and returns the full [100000, 64] float32 output.
"""
import sys

sys.path.insert(0, "/opt/trn_rl_repo")

import numpy as np
import ml_dtypes

BF16 = ml_dtypes.bfloat16


class Cfg:
    def __init__(self, n=100000, e=1600000, nsb=13, sb_j=8,
                 d_in=512, d_h=128, d_out=64, gather_single_packet=False,
                 layers=3, xform=True):
        self.N = n
        self.E = e
        self.NC = 8
        self.NSB = nsb              # super-blocks per core
        self.SB_J = sb_j            # blocks per super-block
        self.NBLK = nsb * sb_j      # blocks per core
        self.R = self.NBLK * 128    # rows per core
        self.N_PAD = self.NC * self.R
        assert self.N_PAD % 4 == 0
        self.CHUNK = self.N_PAD // 4
        assert self.CHUNK <= 32767, "int16 gather index range"
        assert self.R >= (n + self.NC - 1) // self.NC
        self.D_IN = d_in
        self.D_H = d_h
        self.D_OUT = d_out
        self.SP = gather_single_packet
        self.LAYERS = layers
        self.XFORM = xform


DEFAULT_CFG = Cfg()


# ------------------------------------------------------------------ planning
def _node_assignment(adj_rows, cfg):
    """new_id[orig] -> padded id.  Degree-balanced + degree-sorted."""
    deg = np.bincount(adj_rows, minlength=cfg.N)
    order = np.argsort(-deg, kind="stable")
    snake = np.concatenate([np.arange(cfg.NC), np.arange(cfg.NC)[::-1]])
    cores_for_pos = snake[np.arange(cfg.N) % (2 * cfg.NC)]
    new_id = np.empty(cfg.N, dtype=np.int64)
    for c in range(cfg.NC):
        nodes = order[cores_for_pos == c]
        new_id[nodes] = c * cfg.R + np.arange(len(nodes))
    return new_id


def build_plan(adj_rows, adj_cols, adj_vals, cfg):
    """Returns (new_id, G_u, schedule, per-core streams).

    G_u[b, q]: uniform group count per (block, chunk).
    schedule: list over (sb, q) in processing order of dicts
      {sb, q, num_idxs, col_off, grp_off, segments=[(b, gs, ge), ...]}
    streams[c]: dict(idx_stream [128, cols] i16, rl_stream [128, G] f32,
                     val_stream [128, G] f32)
    """
    new_id = _node_assignment(adj_rows, cfg)
    dest = new_id[adj_rows]
    src = new_id[adj_cols]
    core = (dest // cfg.R).astype(np.int64)
    local = dest % cfg.R
    block = (local // 128).astype(np.int64)
    row_local = (local % 128).astype(np.float32)
    chunk = (src // cfg.CHUNK).astype(np.int64)
    idx16 = (src % cfg.CHUNK).astype(np.int16)

    counts = np.zeros((cfg.NC, cfg.NBLK, 4), dtype=np.int64)
    np.add.at(counts, (core, block, chunk), 1)
    G_u = np.ceil(counts / 128).astype(np.int64).max(axis=0)  # [NBLK, 4]

    # schedule (same for all cores)
    sb_blocks = {s: [s + j * cfg.NSB for j in range(cfg.SB_J)]
                 for s in range(cfg.NSB)}
    schedule = []
    col_off = 0
    grp_off = 0
    for s in range(cfg.NSB):
        for q in range(4):
            segments = []
            cur = 0
            for b in sb_blocks[s]:
                g = int(G_u[b, q])
                if g:
                    segments.append((b, cur, cur + g))
                    cur += g
            if cur == 0:
                continue
            num = cur * 128
            schedule.append(dict(sb=s, q=q, num_idxs=num, col_off=col_off,
                                 grp_off=grp_off, segments=segments))
            col_off += num // 16
            grp_off += cur
    total_cols = col_off
    total_groups = grp_off

    # per-core streams
    sb_of_block = np.arange(cfg.NBLK) % cfg.NSB
    streams = []
    for c in range(cfg.NC):
        m = core == c
        b_e = block[m]; rl_e = row_local[m]; q_e = chunk[m]
        ix_e = idx16[m]; v_e = adj_vals[m].astype(np.float32)
        order_e = np.lexsort((rl_e, b_e, q_e, sb_of_block[b_e]))
        b_e = b_e[order_e]; rl_e = rl_e[order_e]; q_e = q_e[order_e]
        ix_e = ix_e[order_e]; v_e = v_e[order_e]

        ix_slots = np.zeros(total_groups * 128, np.int16)
        rl_slots = np.full(total_groups * 128, -1.0, np.float32)
        v_slots = np.zeros(total_groups * 128, np.float32)
        # each (sb, q, block) run lands at its schedule slot offset
        keys = (sb_of_block[b_e] * 8 + q_e) * cfg.NBLK + b_e
        uniq, starts, cnts = np.unique(keys, return_index=True,
                                       return_counts=True)
        run_of_key = {}
        for g in schedule:
            for b, gs, ge in g["segments"]:
                k = (g["sb"] * 8 + g["q"]) * cfg.NBLK + b
                run_of_key[k] = (g["grp_off"] + gs) * 128
        for k, st, cn in zip(uniq, starts, cnts):
            slot0 = run_of_key[int(k)]
            sl = slice(slot0, slot0 + cn)
            ix_slots[sl] = ix_e[st:st + cn]
            rl_slots[sl] = rl_e[st:st + cn]
            v_slots[sl] = v_e[st:st + cn]

        # idx layout per gather: [128, num/16] idx j -> [j%16, j//16], x8
        idx_cols = np.empty((128, total_cols), np.int16)
        for g in schedule:
            n = g["num_idxs"]
            seg = ix_slots[g["grp_off"] * 128: g["grp_off"] * 128 + n]
            tile16 = seg.reshape(n // 16, 16).T          # [16, n/16]
            idx_cols[:, g["col_off"]: g["col_off"] + n // 16] = np.tile(
                tile16, (8, 1))
        rl_stream = rl_slots.reshape(total_groups, 128).T
        val_stream = v_slots.reshape(total_groups, 128).T
        streams.append(dict(
            idx_stream=np.ascontiguousarray(idx_cols),
            rl_stream=np.ascontiguousarray(rl_stream),
            val_stream=np.ascontiguousarray(val_stream)))

    meta = dict(total_cols=total_cols, total_groups=total_groups,
                G_u=G_u, schedule=schedule, sb_blocks=sb_blocks)
    return new_id, meta, streams


# ------------------------------------------------------------ device program
def build_program(meta, cfg):
    from concourse import bacc, mybir, tile
    from concourse.masks import make_identity

    f32 = mybir.dt.float32
    bf16 = mybir.dt.bfloat16
    i16 = mybir.dt.int16
    AF = mybir.ActivationFunctionType
    ALU = mybir.AluOpType

    schedule = meta["schedule"]
    sb_blocks = meta["sb_blocks"]
    total_cols = meta["total_cols"]
    total_groups = meta["total_groups"]
    NSB, SB_J, NBLK = cfg.NSB, cfg.SB_J, cfg.NBLK
    R, N_PAD, CHUNK = cfg.R, cfg.N_PAD, cfg.CHUNK
    D_IN, D_H, D_OUT = cfg.D_IN, cfg.D_H, cfg.D_OUT
    KI = D_IN // 128

    g_cap = max(g["num_idxs"] // 128 for g in schedule)
    def sb_sched(s):
        return [g for g in schedule if g["sb"] == s]
    sbc_cap = max(sum(g["num_idxs"] // 16 for g in sb_sched(s))
                  for s in range(NSB))
    sbg_cap = max(sum(g["num_idxs"] // 128 for g in sb_sched(s))
                  for s in range(NSB))
    # blocks with any edges (uniform over cores)
    live_blocks = {b for g in schedule for (b, _, _) in g["segments"]}

    nc = bacc.Bacc("TRN2", target_bir_lowering=False)

    xT_in = nc.declare_dram_parameter("xT", [D_IN, R], bf16, isOutput=False)
    w_in_p = nc.declare_dram_parameter("w_in", [128, KI, D_H], bf16,
                                       isOutput=False)
    w_hid_p = nc.declare_dram_parameter("w_hid", [128, D_H], bf16,
                                        isOutput=False)
    w_out_p = nc.declare_dram_parameter("w_out", [128, D_OUT], bf16,
                                        isOutput=False)
    iota_p = nc.declare_dram_parameter("iota", [128, 128], bf16,
                                       isOutput=False)
    idx_p = nc.declare_dram_parameter("idxs", [128, total_cols], i16,
                                      isOutput=False)
    rl_p = nc.declare_dram_parameter("rl", [128, total_groups], f32,
                                     isOutput=False)
    val_p = nc.declare_dram_parameter("val", [128, total_groups], f32,
                                      isOutput=False)
    y_out = nc.declare_dram_parameter("y", [R, D_OUT], f32, isOutput=True)

    with tile.TileContext(nc) as tc:
        with tc.tile_pool(name="dram", bufs=1, space="DRAM") as dramp, \
             tc.tile_pool(name="const", bufs=1) as constp, \
             tc.tile_pool(name="hbuf", bufs=1) as hbufp:
            shards = [dramp.tile([R, D_H], bf16, name=f"shard{l}")
                      for l in range(3)]
            tables = [dramp.tile([N_PAD, D_H], bf16, name=f"table{l}",
                                 addr_space="Shared") for l in range(3)]

            w_in_t = constp.tile([128, KI, D_H], bf16)
            nc.sync.dma_start(out=w_in_t[:], in_=w_in_p[:])
            w_hid_t = constp.tile([128, D_H], bf16)
            nc.sync.dma_start(out=w_hid_t[:], in_=w_hid_p[:])
            w_out_t = constp.tile([128, D_OUT], bf16)
            nc.sync.dma_start(out=w_out_t[:], in_=w_out_p[:])
            iota_t = constp.tile([128, 128], bf16)
            nc.sync.dma_start(out=iota_t[:], in_=iota_p[:])
            ident_t = constp.tile([128, 128], f32)
            make_identity(nc, ident_t[:])
            h_buf = hbufp.tile([128, NBLK, D_H], f32)

            # ------------- phase 1: shard of table1 = bf16(x @ W_in)
            with tc.tile_pool(name="dense", bufs=3) as densep, \
                 tc.tile_pool(name="dpsum", bufs=4, space="PSUM") as dpsp:
                for t in range(NBLK):
                    xt = densep.tile([128, KI, 128], bf16, tag="xt")
                    for j in range(KI):
                        nc.sync.dma_start(
                            out=xt[:, j, :],
                            in_=xT_in[j * 128:(j + 1) * 128,
                                      t * 128:(t + 1) * 128])
                    ps = dpsp.tile([128, D_H], f32, space="PSUM", tag="dps")
                    for j in range(KI):
                        nc.tensor.matmul(ps[:], lhsT=xt[:, j, :],
                                         rhs=w_in_t[:, j, :],
                                         start=(j == 0), stop=(j == KI - 1))
                    st = densep.tile([128, D_H], bf16, tag="stage")
                    nc.scalar.activation(st[:], ps[:], AF.Copy)
                    nc.sync.dma_start(
                        out=shards[0][t * 128:(t + 1) * 128, :], in_=st[:])

            nc.gpsimd.collective_compute(
                "AllGather", ALU.bypass, ins=[shards[0][:]],
                outs=[tables[0][:]], replica_groups=[list(range(cfg.NC))])

            # strided views for batched per-SB stores:
            # row (s + j*NSB)*128 + p  <- stage[p, j, :]
            shard_v = [shards[l].rearrange("(j s p) n -> s p j n",
                                           j=SB_J, s=NSB, p=128)
                       for l in range(3)]
            y_v = y_out.rearrange("(j s p) n -> s p j n", j=SB_J, s=NSB,
                                  p=128)

            # ------------- phases 2-4: spmm layers
            for layer in range(cfg.LAYERS):
                table = tables[layer]
                d_l = D_H if layer < 2 else D_OUT
                with tc.tile_pool(name=f"gt{layer}", bufs=8) as gtp, \
                     tc.tile_pool(name=f"wk{layer}", bufs=4) as wkp, \
                     tc.tile_pool(name=f"sg{layer}", bufs=2) as sgp, \
                     tc.tile_pool(name=f"str{layer}", bufs=2) as strp, \
                     tc.tile_pool(name=f"ac{layer}", bufs=4,
                                  space="PSUM") as psp, \
                     tc.tile_pool(name=f"tp{layer}", bufs=2,
                                  space="PSUM") as tpsp:
                    for s in range(NSB):
                        sb_gs = sb_sched(s)
                        if not sb_gs:
                            continue
                        c0 = sb_gs[0]["col_off"]
                        ncols = sum(g["num_idxs"] // 16 for g in sb_gs)
                        g0 = sb_gs[0]["grp_off"]
                        ngrp = sum(g["num_idxs"] // 128 for g in sb_gs)
                        idx_t = strp.tile([128, sbc_cap], i16, tag="idx")
                        nc.sync.dma_start(out=idx_t[:, :ncols],
                                          in_=idx_p[:, c0:c0 + ncols])
                        rl_t = strp.tile([128, sbg_cap], f32, tag="rl")
                        nc.sync.dma_start(out=rl_t[:, :ngrp],
                                          in_=rl_p[:, g0:g0 + ngrp])
                        val_t = strp.tile([128, sbg_cap], f32, tag="val")
                        nc.sync.dma_start(out=val_t[:, :ngrp],
                                          in_=val_p[:, g0:g0 + ngrp])

                        blocks = [b for b in sb_blocks[s] if b in live_blocks]

                        gtiles = []
                        for g in sb_gs:
                            ng = g["num_idxs"] // 128
                            gt = gtp.tile([128, g_cap, 128], bf16, tag="g")
                            nc.gpsimd.dma_gather(
                                out_ap=gt[:, :ng, :],
                                in_ap=table[g["q"] * CHUNK:
                                            (g["q"] + 1) * CHUNK, :],
                                idxs_ap=idx_t[:, g["col_off"] - c0:
                                              g["col_off"] - c0
                                              + g["num_idxs"] // 16],
                                num_idxs=g["num_idxs"],
                                num_idxs_reg=g["num_idxs"],
                                elem_size=D_H,
                                single_packet=cfg.SP)
                            gtiles.append(gt)

                        # block-major accumulation: one PSUM bank per block
                        acc_tiles = {}
                        for b in blocks:
                            segs = []
                            for gt, g in zip(gtiles, sb_gs):
                                for bb, gs, ge in g["segments"]:
                                    if bb == b:
                                        segs.append((gt, g, gs, ge))
                            n_seg_groups = sum(ge - gs for _, _, gs, ge in segs)
                            acc = psp.tile([128, 128], f32, space="PSUM",
                                           tag="acc")
                            acc_tiles[b] = acc
                            done = 0
                            for gt, g, gs, ge in segs:
                                for grp in range(gs, ge):
                                    gg = g["grp_off"] - g0 + grp
                                    s_t = sgp.tile([128, 128], bf16,
                                                   tag="sel")
                                    nc.vector.tensor_scalar(
                                        out=s_t[:], in0=iota_t[:],
                                        scalar1=rl_t[:, gg:gg + 1],
                                        scalar2=val_t[:, gg:gg + 1],
                                        op0=ALU.is_equal, op1=ALU.mult)
                                    nc.tensor.matmul(
                                        acc[:, :d_l], lhsT=s_t[:],
                                        rhs=gt[:, grp, :d_l],
                                        start=(done == 0),
                                        stop=(done == n_seg_groups - 1))
                                    done += 1

                        # epilogue (+ transform feeding next table)
                        if layer < 2:
                            stage = wkp.tile([128, SB_J, D_H], bf16,
                                             tag="tstage")
                            nc.vector.memset(stage[:], 0)
                        else:
                            stage_y = wkp.tile([128, SB_J, D_OUT], f32,
                                               tag="ystage")
                            nc.vector.memset(stage_y[:], 0)
                        for j, b in enumerate(sb_blocks[s]):
                            real = (b * 128) < cfg.N  # any real rows?
                            if b in acc_tiles:
                                acc = acc_tiles[b]
                                if layer == 0:
                                    nc.scalar.activation(h_buf[:, b, :],
                                                         acc[:], AF.Relu)
                                elif layer == 1:
                                    tmp = wkp.tile([128, D_H], f32,
                                                   tag="tmp")
                                    nc.scalar.activation(tmp[:], acc[:],
                                                         AF.Relu)
                                    nc.vector.tensor_tensor(
                                        out=h_buf[:, b, :], in0=tmp[:],
                                        in1=h_buf[:, b, :], op=ALU.add)
                                else:
                                    nc.vector.tensor_copy(
                                        stage_y[:, j, :],
                                        acc[:, :D_OUT])
                            elif real:
                                if layer == 0:
                                    nc.vector.memset(h_buf[:, b, :], 0)
                                elif layer == 2:
                                    pass  # stage_y already zero
                            else:
                                continue  # fully fake block
                            if cfg.XFORM and layer < 2 and (b in acc_tiles or real):
                                w_next = w_hid_t if layer == 0 else w_out_t
                                d_n = D_H if layer == 0 else D_OUT
                                tp = tpsp.tile([128, 128], f32, space="PSUM",
                                               tag="tp")
                                nc.tensor.transpose(tp[:], h_buf[:, b, :],
                                                    ident_t[:])
                                hT = wkp.tile([128, 128], bf16, tag="hT")
                                nc.vector.tensor_copy(hT[:], tp[:])
                                tp2 = tpsp.tile([128, 128], f32,
                                                space="PSUM", tag="tp2")
                                nc.tensor.matmul(tp2[:, :d_n], lhsT=hT[:],
                                                 rhs=w_next[:, :d_n],
                                                 start=True, stop=True)
                                nc.scalar.activation(stage[:, j, :d_n],
                                                     tp2[:, :d_n], AF.Copy)
                        if layer < 2:
                            nc.sync.dma_start(out=shard_v[layer + 1][s],
                                              in_=stage[:])
                        else:
                            nc.sync.dma_start(out=y_v[s], in_=stage_y[:])
                    if cfg.XFORM and layer < 2:
                        nc.gpsimd.collective_compute(
                            "AllGather", ALU.bypass,
                            ins=[shards[layer + 1][:]],
                            outs=[tables[layer + 1][:]],
                            replica_groups=[list(range(cfg.NC))])

    nc.compile()
    return nc


# ------------------------------------------------------------------- driver
def prepare_inputs(x, W_in, W_hid, W_out, new_id, streams, cfg):
    """Build per-core in_maps."""
    KI = cfg.D_IN // 128
    x_pad = np.zeros((cfg.N_PAD, cfg.D_IN), np.float32)
    x_pad[new_id] = np.asarray(x, np.float32)
    xT = np.ascontiguousarray(x_pad.T).astype(BF16)     # [D_IN, N_PAD]

    w_in_t = np.asarray(W_in, np.float32).reshape(KI, 128, cfg.D_H)
    w_in_t = np.ascontiguousarray(w_in_t.transpose(1, 0, 2)).astype(BF16)
    w_hid_t = np.asarray(W_hid, np.float32).astype(BF16)
    w_out_t = np.asarray(W_out, np.float32).astype(BF16)
    iota = np.tile(np.arange(128, dtype=np.float32), (128, 1)).astype(BF16)

    in_maps = []
    for c in range(cfg.NC):
        st = streams[c]
        in_maps.append({
            "xT": np.ascontiguousarray(xT[:, c * cfg.R:(c + 1) * cfg.R]),
            "w_in": w_in_t, "w_hid": w_hid_t, "w_out": w_out_t,
            "iota": iota,
            "idxs": st["idx_stream"].astype(np.int16),
            "rl": st["rl_stream"].astype(np.float32),
            "val": st["val_stream"].astype(np.float32),
        })
    return in_maps


def assemble_output(results, new_id, cfg):
    y_pad = np.concatenate([results[c]["y"] for c in range(cfg.NC)], axis=0)
    return np.ascontiguousarray(y_pad[new_id]).astype(np.float32)


_CACHE = {}


def run(x, adj_rows, adj_cols, adj_vals, W_in, W_hid, W_out,
        cfg=DEFAULT_CFG, trace=False):
    from concourse.bass_utils import run_bass_kernel_spmd
    adj_rows = np.asarray(adj_rows)
    adj_cols = np.asarray(adj_cols)
    adj_vals = np.asarray(adj_vals, np.float32)
    key = ("plan", adj_rows.tobytes()[:64], cfg.N, cfg.E, cfg.NBLK)
    if key not in _CACHE:
        new_id, meta, streams = build_plan(adj_rows, adj_cols, adj_vals, cfg)
        nc = build_program(meta, cfg)
        _CACHE[key] = (new_id, meta, streams, nc)
    new_id, meta, streams, nc = _CACHE[key]
    in_maps = prepare_inputs(x, W_in, W_hid, W_out, new_id, streams, cfg)
    kw = {}
    if trace:
        try:
            import ntff_hook
            ntff_hook.install()
            kw["trace"] = True
        except Exception:
            pass
    res = run_bass_kernel_spmd(nc, in_maps, list(range(cfg.NC)), **kw)
    out = assemble_output(res.results, new_id, cfg)
    return out, res


def kernel(x, adj_rows, adj_cols, adj_vals, W_in, W_hid, W_out):
    out, _ = run(x, adj_rows, adj_cols, adj_vals, W_in, W_hid, W_out)
    return out



# revision 15
# speedup vs baseline: 2.3148x; 2.3148x over previous
"""GCN (3-layer graph conv) Trainium2 kernel running SPMD on 8 NeuronCores.

Approach
--------
- Destination-node 1D sharding; nodes renumbered (degree-balanced snake
  deal across cores, degree-sorted within a core) and padded so each core
  owns R rows = NBLK blocks of 128.
- Per layer a bf16 feature table ([N_PAD, 128] in HBM, replicated per
  core via AllGather of per-core transform shards) is randomly gathered
  with gpsimd.dma_gather (int16 indices -> 4 source chunks of N_PAD/4
  rows each).
- SpMM: for each 128-row destination block, one-hot selection matrices
  (DVE tensor_scalar: is_equal(iota, row_local) * val) x gathered tiles
  (PE matmul) accumulate the block result in PSUM.
- The schedule is UNIFORM across cores (group counts per (block, chunk)
  padded to the max over cores) so one SPMD program serves all 8 cores;
  per-core behavior differs only through the idx/row-local/val streams.

kernel(**inputs) accepts the full-size inputs from reference.setup_inputs
and returns the full [100000, 64] float32 output.
"""
import sys

sys.path.insert(0, "/opt/trn_rl_repo")

import numpy as np
import ml_dtypes

BF16 = ml_dtypes.bfloat16


class Cfg:
    def __init__(self, n=100000, e=1600000, nsb=13, sb_j=8,
                 d_in=512, d_h=128, d_out=64, gather_single_packet=False,
                 layers=3, xform=True, n_queues=4):
        self.N = n
        self.E = e
        self.NC = 8
        self.NSB = nsb              # super-blocks per core
        self.SB_J = sb_j            # blocks per super-block
        self.NBLK = nsb * sb_j      # blocks per core
        self.R = self.NBLK * 128    # rows per core
        self.N_PAD = self.NC * self.R
        assert self.N_PAD % 4 == 0
        self.CHUNK = self.N_PAD // 4
        assert self.CHUNK <= 32767, "int16 gather index range"
        assert self.R >= (n + self.NC - 1) // self.NC
        self.D_IN = d_in
        self.D_H = d_h
        self.D_OUT = d_out
        self.SP = gather_single_packet
        self.LAYERS = layers
        self.XFORM = xform
        self.NQ = n_queues


DEFAULT_CFG = Cfg()


# ------------------------------------------------------------------ planning
def _node_assignment(adj_rows, cfg):
    """new_id[orig] -> padded id.  Degree-balanced + degree-sorted."""
    deg = np.bincount(adj_rows, minlength=cfg.N)
    order = np.argsort(-deg, kind="stable")
    snake = np.concatenate([np.arange(cfg.NC), np.arange(cfg.NC)[::-1]])
    cores_for_pos = snake[np.arange(cfg.N) % (2 * cfg.NC)]
    new_id = np.empty(cfg.N, dtype=np.int64)
    for c in range(cfg.NC):
        nodes = order[cores_for_pos == c]
        new_id[nodes] = c * cfg.R + np.arange(len(nodes))
    return new_id


def build_plan(adj_rows, adj_cols, adj_vals, cfg):
    """Returns (new_id, G_u, schedule, per-core streams).

    G_u[b, q]: uniform group count per (block, chunk).
    schedule: list over (sb, q) in processing order of dicts
      {sb, q, num_idxs, col_off, grp_off, segments=[(b, gs, ge), ...]}
    streams[c]: dict(idx_stream [128, cols] i16, rl_stream [128, G] f32,
                     val_stream [128, G] f32)
    """
    new_id = _node_assignment(adj_rows, cfg)
    dest = new_id[adj_rows]
    src = new_id[adj_cols]
    core = (dest // cfg.R).astype(np.int64)
    local = dest % cfg.R
    block = (local // 128).astype(np.int64)
    row_local = (local % 128).astype(np.float32)
    chunk = (src // cfg.CHUNK).astype(np.int64)
    idx16 = (src % cfg.CHUNK).astype(np.int16)

    counts = np.zeros((cfg.NC, cfg.NBLK, 4), dtype=np.int64)
    np.add.at(counts, (core, block, chunk), 1)
    G_u = np.ceil(counts / 128).astype(np.int64).max(axis=0)  # [NBLK, 4]
    mean_cnt = counts.mean(axis=0)  # [NBLK, 4]

    # schedule (same for all cores); most-padded block last per (s, q) so
    # its pad slots are trailing in the gather stream (idx=-1 -> dropped)
    sb_blocks = {s: [s + j * cfg.NSB for j in range(cfg.SB_J)]
                 for s in range(cfg.NSB)}
    schedule = []
    col_off = 0
    grp_off = 0
    for s in range(cfg.NSB):
        for q in range(4):
            order = sorted(sb_blocks[s],
                           key=lambda b: G_u[b, q] * 128 - mean_cnt[b, q])
            segments = []
            cur = 0
            for b in order:
                g = int(G_u[b, q])
                if g:
                    segments.append((b, cur, cur + g))
                    cur += g
            if cur == 0:
                continue
            num = cur * 128
            schedule.append(dict(sb=s, q=q, num_idxs=num, col_off=col_off,
                                 grp_off=grp_off, segments=segments))
            col_off += num // 16
            grp_off += cur
    total_cols = col_off
    total_groups = grp_off

    # per-core streams
    sb_of_block = np.arange(cfg.NBLK) % cfg.NSB
    streams = []
    for c in range(cfg.NC):
        m = core == c
        b_e = block[m]; rl_e = row_local[m]; q_e = chunk[m]
        ix_e = idx16[m]; v_e = adj_vals[m].astype(np.float32)
        order_e = np.lexsort((rl_e, b_e, q_e, sb_of_block[b_e]))
        b_e = b_e[order_e]; rl_e = rl_e[order_e]; q_e = q_e[order_e]
        ix_e = ix_e[order_e]; v_e = v_e[order_e]

        ix_slots = np.zeros(total_groups * 128, np.int16)
        rl_slots = np.full(total_groups * 128, -1.0, np.float32)
        v_slots = np.zeros(total_groups * 128, np.float32)
        # each (sb, q, block) run lands at its schedule slot offset
        keys = (sb_of_block[b_e] * 8 + q_e) * cfg.NBLK + b_e
        uniq, starts, cnts = np.unique(keys, return_index=True,
                                       return_counts=True)
        run_of_key = {}
        for g in schedule:
            for b, gs, ge in g["segments"]:
                k = (g["sb"] * 8 + g["q"]) * cfg.NBLK + b
                run_of_key[k] = (g["grp_off"] + gs) * 128
        for k, st, cn in zip(uniq, starts, cnts):
            slot0 = run_of_key[int(k)]
            sl = slice(slot0, slot0 + cn)
            ix_slots[sl] = ix_e[st:st + cn]
            rl_slots[sl] = rl_e[st:st + cn]
            v_slots[sl] = v_e[st:st + cn]

        # NOTE: trailing idx=-1 dropping requires num_idxs_reg to carry the
        # per-core post-drop count (ring bookkeeping at decode advances by
        # the register count; a mismatch desyncs the descriptor ring and
        # hangs the device). Pads keep idx 0 until that is wired up.
        call_counts = []
        for g in schedule:
            a = g["grp_off"] * 128
            e_ = a + g["num_idxs"]
            real = np.nonzero(rl_slots[a:e_] >= 0)[0]
            call_counts.append(int(real[-1]) + 1 if len(real) else 0)
        call_counts = np.asarray(call_counts, np.int32)

        # idx layout per gather: [128, num/16] idx j -> [j%16, j//16], x8
        idx_cols = np.empty((128, total_cols), np.int16)
        for g in schedule:
            n = g["num_idxs"]
            seg = ix_slots[g["grp_off"] * 128: g["grp_off"] * 128 + n]
            tile16 = seg.reshape(n // 16, 16).T          # [16, n/16]
            idx_cols[:, g["col_off"]: g["col_off"] + n // 16] = np.tile(
                tile16, (8, 1))
        rl_stream = rl_slots.reshape(total_groups, 128).T
        val_stream = v_slots.reshape(total_groups, 128).T
        streams.append(dict(
            idx_stream=np.ascontiguousarray(idx_cols),
            rl_stream=np.ascontiguousarray(rl_stream),
            val_stream=np.ascontiguousarray(val_stream),
            call_counts=call_counts))

    meta = dict(total_cols=total_cols, total_groups=total_groups,
                G_u=G_u, schedule=schedule, sb_blocks=sb_blocks)
    return new_id, meta, streams


# ------------------------------------------------------------ device program
def build_program(meta, cfg):
    from concourse import bacc, mybir, tile
    from concourse.masks import make_identity

    f32 = mybir.dt.float32
    bf16 = mybir.dt.bfloat16
    i16 = mybir.dt.int16
    AF = mybir.ActivationFunctionType
    ALU = mybir.AluOpType

    schedule = meta["schedule"]
    sb_blocks = meta["sb_blocks"]
    total_cols = meta["total_cols"]
    total_groups = meta["total_groups"]
    NSB, SB_J, NBLK = cfg.NSB, cfg.SB_J, cfg.NBLK
    R, N_PAD, CHUNK = cfg.R, cfg.N_PAD, cfg.CHUNK
    D_IN, D_H, D_OUT = cfg.D_IN, cfg.D_H, cfg.D_OUT
    KI = D_IN // 128

    g_cap = max(g["num_idxs"] // 128 for g in schedule)
    def sb_sched(s):
        return [g for g in schedule if g["sb"] == s]
    sbc_cap = max(sum(g["num_idxs"] // 16 for g in sb_sched(s))
                  for s in range(NSB))
    sbg_cap = max(sum(g["num_idxs"] // 128 for g in sb_sched(s))
                  for s in range(NSB))
    # blocks with any edges (uniform over cores)
    live_blocks = {b for g in schedule for (b, _, _) in g["segments"]}

    nc = bacc.Bacc("TRN2", target_bir_lowering=False,
                   num_swdge_queues=cfg.NQ)

    xT_in = nc.declare_dram_parameter("xT", [D_IN, R], bf16, isOutput=False)
    w_in_p = nc.declare_dram_parameter("w_in", [128, KI, D_H], bf16,
                                       isOutput=False)
    w_hid_p = nc.declare_dram_parameter("w_hid", [128, D_H], bf16,
                                        isOutput=False)
    w_out_p = nc.declare_dram_parameter("w_out", [128, D_OUT], bf16,
                                        isOutput=False)
    iota_p = nc.declare_dram_parameter("iota", [128, 128], bf16,
                                       isOutput=False)
    idx_p = nc.declare_dram_parameter("idxs", [128, total_cols], i16,
                                      isOutput=False)
    rl_p = nc.declare_dram_parameter("rl", [128, total_groups], bf16,
                                     isOutput=False)
    val_p = nc.declare_dram_parameter("val", [128, total_groups], bf16,
                                      isOutput=False)
    y_out = nc.declare_dram_parameter("y", [R, D_OUT], f32, isOutput=True)

    with tile.TileContext(nc) as tc:
        with tc.tile_pool(name="dram", bufs=1, space="DRAM") as dramp, \
             tc.tile_pool(name="const", bufs=1) as constp, \
             tc.tile_pool(name="hbuf", bufs=1) as hbufp:
            shards = [dramp.tile([R, D_H], bf16, name=f"shard{l}")
                      for l in range(3)]
            tables = [dramp.tile([N_PAD, D_H], bf16, name=f"table{l}",
                                 addr_space="Shared") for l in range(3)]

            w_in_t = constp.tile([128, KI, D_H], bf16)
            nc.sync.dma_start(out=w_in_t[:], in_=w_in_p[:])
            w_hid_t = constp.tile([128, D_H], bf16)
            nc.sync.dma_start(out=w_hid_t[:], in_=w_hid_p[:])
            w_out_t = constp.tile([128, D_OUT], bf16)
            nc.sync.dma_start(out=w_out_t[:], in_=w_out_p[:])
            iota_t = constp.tile([128, 128], bf16)
            nc.sync.dma_start(out=iota_t[:], in_=iota_p[:])
            ident_t = constp.tile([128, 128], f32)
            make_identity(nc, ident_t[:])
            h_buf = hbufp.tile([128, NBLK, D_H], f32)

            # ------------- phase 1: shard of table1 = bf16(x @ W_in)
            with tc.tile_pool(name="dense", bufs=3) as densep, \
                 tc.tile_pool(name="dpsum", bufs=4, space="PSUM") as dpsp:
                for t in range(NBLK):
                    xt = densep.tile([128, KI, 128], bf16, tag="xt")
                    for j in range(KI):
                        nc.sync.dma_start(
                            out=xt[:, j, :],
                            in_=xT_in[j * 128:(j + 1) * 128,
                                      t * 128:(t + 1) * 128])
                    ps = dpsp.tile([128, D_H], f32, space="PSUM", tag="dps")
                    for j in range(KI):
                        nc.tensor.matmul(ps[:], lhsT=xt[:, j, :],
                                         rhs=w_in_t[:, j, :],
                                         start=(j == 0), stop=(j == KI - 1))
                    st = densep.tile([128, D_H], bf16, tag="stage")
                    nc.scalar.activation(st[:], ps[:], AF.Copy)
                    nc.sync.dma_start(
                        out=shards[0][t * 128:(t + 1) * 128, :], in_=st[:])

            nc.gpsimd.collective_compute(
                "AllGather", ALU.bypass, ins=[shards[0][:]],
                outs=[tables[0][:]], replica_groups=[list(range(cfg.NC))])

            # strided views for batched per-SB stores:
            # row (s + j*NSB)*128 + p  <- stage[p, j, :]
            shard_v = [shards[l].rearrange("(j s p) n -> s p j n",
                                           j=SB_J, s=NSB, p=128)
                       for l in range(3)]
            y_v = y_out.rearrange("(j s p) n -> s p j n", j=SB_J, s=NSB,
                                  p=128)

            # ------------- phases 2-4: spmm layers
            for layer in range(cfg.LAYERS):
                table = tables[layer]
                d_l = D_H if layer < 2 else D_OUT
                with tc.tile_pool(name=f"gt{layer}", bufs=5) as gtp, \
                     tc.tile_pool(name=f"wk{layer}", bufs=4) as wkp, \
                     tc.tile_pool(name=f"sg{layer}", bufs=5) as sgp, \
                     tc.tile_pool(name=f"str{layer}", bufs=2) as strp, \
                     tc.tile_pool(name=f"ac{layer}", bufs=4,
                                  space="PSUM") as psp, \
                     tc.tile_pool(name=f"tp{layer}", bufs=2,
                                  space="PSUM") as tpsp:
                    for s in range(NSB):
                        sb_gs = sb_sched(s)
                        if not sb_gs:
                            continue
                        c0 = sb_gs[0]["col_off"]
                        ncols = sum(g["num_idxs"] // 16 for g in sb_gs)
                        g0 = sb_gs[0]["grp_off"]
                        ngrp = sum(g["num_idxs"] // 128 for g in sb_gs)
                        idx_t = strp.tile([128, sbc_cap], i16, tag="idx")
                        nc.sync.dma_start(out=idx_t[:, :ncols],
                                          in_=idx_p[:, c0:c0 + ncols])
                        rl_t = strp.tile([128, sbg_cap], bf16, tag="rl")
                        nc.sync.dma_start(out=rl_t[:, :ngrp],
                                          in_=rl_p[:, g0:g0 + ngrp])
                        val_t = strp.tile([128, sbg_cap], bf16, tag="val")
                        nc.sync.dma_start(out=val_t[:, :ngrp],
                                          in_=val_p[:, g0:g0 + ngrp])

                        blocks = [b for b in sb_blocks[s] if b in live_blocks]

                        # all 4 chunk gathers in flight on 4 SWDGE queues
                        gtiles = []
                        for g in sb_gs:
                            ng = g["num_idxs"] // 128
                            gt = gtp.tile([128, g_cap, 128], bf16, tag="g")
                            nc.gpsimd.dma_gather(
                                out_ap=gt[:, :ng, :],
                                in_ap=table[g["q"] * CHUNK:
                                            (g["q"] + 1) * CHUNK, :],
                                idxs_ap=idx_t[:, g["col_off"] - c0:
                                              g["col_off"] - c0
                                              + g["num_idxs"] // 16],
                                num_idxs=g["num_idxs"],
                                num_idxs_reg=g["num_idxs"],
                                elem_size=D_H,
                                single_packet=cfg.SP,
                                queue_num=g["q"] % cfg.NQ)
                            gtiles.append(gt)

                        # sel batches per chunk (DVE, overlaps the gathers)
                        stiles = []
                        for g in sb_gs:
                            ng = g["num_idxs"] // 128
                            gl = g["grp_off"] - g0   # local group offset
                            sel = sgp.tile([128, g_cap, 128], bf16,
                                           tag="sel")
                            nc.vector.tensor_tensor(
                                out=sel[:, :ng, :],
                                in0=iota_t[:].unsqueeze(1)
                                    .broadcast_to([128, ng, 128]),
                                in1=rl_t[:, gl:gl + ng].unsqueeze(2)
                                    .broadcast_to([128, ng, 128]),
                                op=ALU.is_equal)
                            nc.vector.tensor_tensor(
                                out=sel[:, :ng, :],
                                in0=sel[:, :ng, :],
                                in1=val_t[:, gl:gl + ng].unsqueeze(2)
                                    .broadcast_to([128, ng, 128]),
                                op=ALU.mult)
                            stiles.append(sel)

                        # block-major accumulation: one PSUM bank per block
                        acc_tiles = {}
                        for b in blocks:
                            segs = []
                            for gt, sel, g in zip(gtiles, stiles, sb_gs):
                                for bb, gs, ge in g["segments"]:
                                    if bb == b:
                                        segs.append((gt, sel, gs, ge))
                            n_seg_groups = sum(ge - gs
                                               for _, _, gs, ge in segs)
                            acc = psp.tile([128, 128], f32, space="PSUM",
                                           tag="acc")
                            acc_tiles[b] = acc
                            done = 0
                            for gt, sel, gs, ge in segs:
                                for grp in range(gs, ge):
                                    nc.tensor.matmul(
                                        acc[:, :d_l],
                                        lhsT=sel[:, grp, :],
                                        rhs=gt[:, grp, :d_l],
                                        start=(done == 0),
                                        stop=(done == n_seg_groups - 1))
                                    done += 1

                        # epilogue (+ transform feeding next table)
                        if layer < 2:
                            stage = wkp.tile([128, SB_J, D_H], bf16,
                                             tag="tstage")
                            nc.vector.memset(stage[:], 0)
                        else:
                            stage_y = wkp.tile([128, SB_J, D_OUT], f32,
                                               tag="ystage")
                            nc.vector.memset(stage_y[:], 0)
                        for j, b in enumerate(sb_blocks[s]):
                            real = (b * 128) < cfg.N  # any real rows?
                            if b in acc_tiles:
                                acc = acc_tiles[b]
                                if layer == 0:
                                    nc.scalar.activation(h_buf[:, b, :],
                                                         acc[:], AF.Relu)
                                elif layer == 1:
                                    tmp = wkp.tile([128, D_H], f32,
                                                   tag="tmp")
                                    nc.scalar.activation(tmp[:], acc[:],
                                                         AF.Relu)
                                    nc.vector.tensor_tensor(
                                        out=h_buf[:, b, :], in0=tmp[:],
                                        in1=h_buf[:, b, :], op=ALU.add)
                                else:
                                    nc.vector.tensor_copy(
                                        stage_y[:, j, :],
                                        acc[:, :D_OUT])
                            elif real:
                                if layer == 0:
                                    nc.vector.memset(h_buf[:, b, :], 0)
                                elif layer == 2:
                                    pass  # stage_y already zero
                            else:
                                continue  # fully fake block
                            if cfg.XFORM and layer < 2 and (b in acc_tiles or real):
                                w_next = w_hid_t if layer == 0 else w_out_t
                                d_n = D_H if layer == 0 else D_OUT
                                tp = tpsp.tile([128, 128], f32, space="PSUM",
                                               tag="tp")
                                nc.tensor.transpose(tp[:], h_buf[:, b, :],
                                                    ident_t[:])
                                hT = wkp.tile([128, 128], bf16, tag="hT")
                                nc.vector.tensor_copy(hT[:], tp[:])
                                tp2 = tpsp.tile([128, 128], f32,
                                                space="PSUM", tag="tp2")
                                nc.tensor.matmul(tp2[:, :d_n], lhsT=hT[:],
                                                 rhs=w_next[:, :d_n],
                                                 start=True, stop=True)
                                nc.scalar.activation(stage[:, j, :d_n],
                                                     tp2[:, :d_n], AF.Copy)
                        if layer < 2:
                            nc.sync.dma_start(out=shard_v[layer + 1][s],
                                              in_=stage[:])
                        else:
                            nc.sync.dma_start(out=y_v[s], in_=stage_y[:])
                    if cfg.XFORM and layer < 2:
                        nc.gpsimd.collective_compute(
                            "AllGather", ALU.bypass,
                            ins=[shards[layer + 1][:]],
                            outs=[tables[layer + 1][:]],
                            replica_groups=[list(range(cfg.NC))])

    nc.compile()
    return nc


# ------------------------------------------------------------------- driver
def prepare_inputs(x, W_in, W_hid, W_out, new_id, streams, cfg):
    """Build per-core in_maps."""
    KI = cfg.D_IN // 128
    x_pad = np.zeros((cfg.N_PAD, cfg.D_IN), np.float32)
    x_pad[new_id] = np.asarray(x, np.float32)
    xT = np.ascontiguousarray(x_pad.T).astype(BF16)     # [D_IN, N_PAD]

    w_in_t = np.asarray(W_in, np.float32).reshape(KI, 128, cfg.D_H)
    w_in_t = np.ascontiguousarray(w_in_t.transpose(1, 0, 2)).astype(BF16)
    w_hid_t = np.asarray(W_hid, np.float32).astype(BF16)
    w_out_t = np.asarray(W_out, np.float32).astype(BF16)
    iota = np.tile(np.arange(128, dtype=np.float32), (128, 1)).astype(BF16)

    in_maps = []
    for c in range(cfg.NC):
        st = streams[c]
        in_maps.append({
            "xT": np.ascontiguousarray(xT[:, c * cfg.R:(c + 1) * cfg.R]),
            "w_in": w_in_t, "w_hid": w_hid_t, "w_out": w_out_t,
            "iota": iota,
            "idxs": st["idx_stream"].astype(np.int16),
            "rl": st["rl_stream"].astype(BF16),
            "val": st["val_stream"].astype(BF16),
        })
    return in_maps


def assemble_output(results, new_id, cfg):
    y_pad = np.concatenate([results[c]["y"] for c in range(cfg.NC)], axis=0)
    return np.ascontiguousarray(y_pad[new_id]).astype(np.float32)


_CACHE = {}


def run(x, adj_rows, adj_cols, adj_vals, W_in, W_hid, W_out,
        cfg=DEFAULT_CFG, trace=False):
    from concourse.bass_utils import run_bass_kernel_spmd
    adj_rows = np.asarray(adj_rows)
    adj_cols = np.asarray(adj_cols)
    adj_vals = np.asarray(adj_vals, np.float32)
    key = ("plan", adj_rows.tobytes()[:64], cfg.N, cfg.E, cfg.NBLK)
    if key not in _CACHE:
        new_id, meta, streams = build_plan(adj_rows, adj_cols, adj_vals, cfg)
        nc = build_program(meta, cfg)
        _CACHE[key] = (new_id, meta, streams, nc)
    new_id, meta, streams, nc = _CACHE[key]
    in_maps = prepare_inputs(x, W_in, W_hid, W_out, new_id, streams, cfg)
    kw = {}
    if trace:
        try:
            import ntff_hook
            ntff_hook.install()
            kw["trace"] = True
        except Exception:
            pass
    res = run_bass_kernel_spmd(nc, in_maps, list(range(cfg.NC)), **kw)
    out = assemble_output(res.results, new_id, cfg)
    return out, res


def kernel(x, adj_rows, adj_cols, adj_vals, W_in, W_hid, W_out):
    out, _ = run(x, adj_rows, adj_cols, adj_vals, W_in, W_hid, W_out)
    return out



# revision 36
# speedup vs baseline: 2.7483x; 1.1873x over previous
"""GCN (3-layer graph conv) Trainium2 kernel running SPMD on 8 NeuronCores.

Approach
--------
- Destination-node 1D sharding; nodes renumbered (degree-balanced snake
  deal across cores, degree-sorted within a core) and padded so each core
  owns R rows = NBLK blocks of 128.
- Per layer a bf16 feature table ([N_PAD, 128] in HBM, replicated per
  core via AllGather of per-core transform shards) is randomly gathered
  with gpsimd.dma_gather (int16 indices -> 4 source chunks of N_PAD/4
  rows each).
- SpMM: for each 128-row destination block, one-hot selection matrices
  (DVE tensor_scalar: is_equal(iota, row_local) * val) x gathered tiles
  (PE matmul) accumulate the block result in PSUM.
- The schedule is UNIFORM across cores (group counts per (block, chunk)
  padded to the max over cores) so one SPMD program serves all 8 cores;
  per-core behavior differs only through the idx/row-local/val streams.

kernel(**inputs) accepts the full-size inputs from reference.setup_inputs
and returns the full [100000, 64] float32 output.
"""
import sys

sys.path.insert(0, "/opt/trn_rl_repo")

import numpy as np
import ml_dtypes

BF16 = ml_dtypes.bfloat16


class Cfg:
    def __init__(self, n=100000, e=1600000, nsb=13, sb_j=8,
                 d_in=512, d_h=128, d_out=64, gather_single_packet=False,
                 layers=3, xform=True, n_queues=4):
        self.N = n
        self.E = e
        self.NC = 8
        self.NSB = nsb              # super-blocks per core
        self.SB_J = sb_j            # blocks per super-block
        self.NBLK = nsb * sb_j      # blocks per core
        self.R = self.NBLK * 128    # rows per core
        self.N_PAD = self.NC * self.R
        assert self.R % (4 * 128) == 0
        self.R4 = self.R // 4       # shard rows per table chunk
        self.BPC = self.R4 // 128   # blocks per chunk
        self.CHUNK = self.NC * self.R4   # rows per chunk table
        assert self.CHUNK <= 32767, "int16 gather index range"
        assert self.R >= (n + self.NC - 1) // self.NC
        self.D_IN = d_in
        self.D_H = d_h
        self.D_OUT = d_out
        self.SP = gather_single_packet
        self.LAYERS = layers
        self.XFORM = xform
        self.NQ = n_queues


DEFAULT_CFG = Cfg()


# ------------------------------------------------------------------ planning
def _node_assignment(adj_rows, cfg):
    """new_id[orig] -> padded id.  Degree-balanced + degree-sorted."""
    deg = np.bincount(adj_rows, minlength=cfg.N)
    order = np.argsort(-deg, kind="stable")
    snake = np.concatenate([np.arange(cfg.NC), np.arange(cfg.NC)[::-1]])
    cores_for_pos = snake[np.arange(cfg.N) % (2 * cfg.NC)]
    new_id = np.empty(cfg.N, dtype=np.int64)
    for c in range(cfg.NC):
        nodes = order[cores_for_pos == c]
        r = np.arange(len(nodes))
        # deal degree-ranked nodes round-robin across blocks: every block
        # gets a balanced degree mix (SBs and table chunks stay uniform)
        pos = (r % cfg.NBLK) * 128 + r // cfg.NBLK
        new_id[nodes] = c * cfg.R + pos
    return new_id


def build_plan(adj_rows, adj_cols, adj_vals, cfg):
    """Returns (new_id, G_u, schedule, per-core streams).

    G_u[b, q]: uniform group count per (block, chunk).
    schedule: list over (sb, q) in processing order of dicts
      {sb, q, num_idxs, col_off, grp_off, segments=[(b, gs, ge), ...]}
    streams[c]: dict(idx_stream [128, cols] i16, rl_stream [128, G] f32,
                     val_stream [128, G] f32)
    """
    new_id = _node_assignment(adj_rows, cfg)
    dest = new_id[adj_rows]
    src = new_id[adj_cols]
    core = (dest // cfg.R).astype(np.int64)
    local = dest % cfg.R
    block = (local // 128).astype(np.int64)
    row_local = (local % 128).astype(np.float32)
    # table chunk c holds every core's shard rows [c*R4, (c+1)*R4)
    src_core = src // cfg.R
    src_loc = src % cfg.R
    chunk = (src_loc // cfg.R4).astype(np.int64)
    idx16 = (src_core * cfg.R4 + src_loc % cfg.R4).astype(np.int16)

    counts = np.zeros((cfg.NC, cfg.NBLK, 4), dtype=np.int64)
    np.add.at(counts, (core, block, chunk), 1)
    G_u = np.ceil(counts / 128).astype(np.int64).max(axis=0)  # [NBLK, 4]
    mean_cnt = counts.mean(axis=0)  # [NBLK, 4]

    # schedule (same for all cores); most-padded block last per (s, q) so
    # its pad slots are trailing in the gather stream (idx=-1 -> dropped)
    sb_blocks = {s: [s * cfg.SB_J + j for j in range(cfg.SB_J)]
                 for s in range(cfg.NSB)}
    schedule = []
    col_off = 0
    grp_off = 0
    for s in range(cfg.NSB):
        for q in range(4):
            order = sorted(sb_blocks[s],
                           key=lambda b: G_u[b, q] * 128 - mean_cnt[b, q])
            segments = []
            cur = 0
            for b in order:
                g = int(G_u[b, q])
                if g:
                    segments.append((b, cur, cur + g))
                    cur += g
            if cur == 0:
                continue
            num = cur * 128
            schedule.append(dict(sb=s, q=q, num_idxs=num, col_off=col_off,
                                 grp_off=grp_off, segments=segments))
            col_off += num // 16
            grp_off += cur
    total_cols = col_off
    total_groups = grp_off

    # per-core streams
    sb_of_block = np.arange(cfg.NBLK) // cfg.SB_J
    streams = []
    for c in range(cfg.NC):
        m = core == c
        b_e = block[m]; rl_e = row_local[m]; q_e = chunk[m]
        ix_e = idx16[m]; v_e = adj_vals[m].astype(np.float32)
        order_e = np.lexsort((rl_e, b_e, q_e, sb_of_block[b_e]))
        b_e = b_e[order_e]; rl_e = rl_e[order_e]; q_e = q_e[order_e]
        ix_e = ix_e[order_e]; v_e = v_e[order_e]

        ix_slots = np.zeros(total_groups * 128, np.int16)
        rl_slots = np.full(total_groups * 128, -1.0, np.float32)
        v_slots = np.zeros(total_groups * 128, np.float32)
        # each (sb, q, block) run lands at its schedule slot offset
        keys = (sb_of_block[b_e] * 8 + q_e) * cfg.NBLK + b_e
        uniq, starts, cnts = np.unique(keys, return_index=True,
                                       return_counts=True)
        run_of_key = {}
        for g in schedule:
            for b, gs, ge in g["segments"]:
                k = (g["sb"] * 8 + g["q"]) * cfg.NBLK + b
                run_of_key[k] = (g["grp_off"] + gs) * 128
        for k, st, cn in zip(uniq, starts, cnts):
            slot0 = run_of_key[int(k)]
            sl = slice(slot0, slot0 + cn)
            ix_slots[sl] = ix_e[st:st + cn]
            rl_slots[sl] = rl_e[st:st + cn]
            v_slots[sl] = v_e[st:st + cn]

        # NOTE: trailing idx=-1 dropping requires num_idxs_reg to carry the
        # per-core post-drop count (ring bookkeeping at decode advances by
        # the register count; a mismatch desyncs the descriptor ring and
        # hangs the device). Pads keep idx 0 until that is wired up.
        call_counts = []
        for g in schedule:
            a = g["grp_off"] * 128
            e_ = a + g["num_idxs"]
            real = np.nonzero(rl_slots[a:e_] >= 0)[0]
            call_counts.append(int(real[-1]) + 1 if len(real) else 0)
        call_counts = np.asarray(call_counts, np.int32)

        # idx layout per gather: [128, num/16] idx j -> [j%16, j//16], x8
        idx_cols = np.empty((128, total_cols), np.int16)
        for g in schedule:
            n = g["num_idxs"]
            seg = ix_slots[g["grp_off"] * 128: g["grp_off"] * 128 + n]
            tile16 = seg.reshape(n // 16, 16).T          # [16, n/16]
            idx_cols[:, g["col_off"]: g["col_off"] + n // 16] = np.tile(
                tile16, (8, 1))
        rl_stream = rl_slots.reshape(total_groups, 128).T
        val_stream = v_slots.reshape(total_groups, 128).T
        streams.append(dict(
            idx_stream=np.ascontiguousarray(idx_cols),
            rl_stream=np.ascontiguousarray(rl_stream),
            val_stream=np.ascontiguousarray(val_stream),
            call_counts=call_counts))

    meta = dict(total_cols=total_cols, total_groups=total_groups,
                G_u=G_u, schedule=schedule, sb_blocks=sb_blocks)
    return new_id, meta, streams


# ------------------------------------------------------------ device program
def build_program(meta, cfg):
    from concourse import bacc, mybir, tile
    from concourse.masks import make_identity

    f32 = mybir.dt.float32
    bf16 = mybir.dt.bfloat16
    i16 = mybir.dt.int16
    AF = mybir.ActivationFunctionType
    ALU = mybir.AluOpType

    schedule = meta["schedule"]
    sb_blocks = meta["sb_blocks"]
    total_cols = meta["total_cols"]
    total_groups = meta["total_groups"]
    NSB, SB_J, NBLK = cfg.NSB, cfg.SB_J, cfg.NBLK
    R, N_PAD, CHUNK = cfg.R, cfg.N_PAD, cfg.CHUNK
    D_IN, D_H, D_OUT = cfg.D_IN, cfg.D_H, cfg.D_OUT
    KI = D_IN // 128

    g_cap = max(g["num_idxs"] // 128 for g in schedule)
    def sb_sched(s):
        return [g for g in schedule if g["sb"] == s]
    sbc_cap = max(sum(g["num_idxs"] // 16 for g in sb_sched(s))
                  for s in range(NSB))
    sbg_cap = max(sum(g["num_idxs"] // 128 for g in sb_sched(s))
                  for s in range(NSB))
    # blocks with any edges (uniform over cores)
    live_blocks = {b for g in schedule for (b, _, _) in g["segments"]}

    nc = bacc.Bacc("TRN2", target_bir_lowering=False,
                   num_swdge_queues=cfg.NQ)

    xT_in = nc.declare_dram_parameter("xT", [NBLK, 128, KI, 128], bf16,
                                      isOutput=False)
    w_in_p = nc.declare_dram_parameter("w_in", [128, KI, D_H], bf16,
                                       isOutput=False)
    w_hid_p = nc.declare_dram_parameter("w_hid", [128, D_H], bf16,
                                        isOutput=False)
    w_out_p = nc.declare_dram_parameter("w_out", [128, D_OUT], bf16,
                                        isOutput=False)
    iota_p = nc.declare_dram_parameter("iota", [128, 128], bf16,
                                       isOutput=False)
    idx_p = nc.declare_dram_parameter("idxs", [128, total_cols], i16,
                                      isOutput=False)
    rl_p = nc.declare_dram_parameter("rl", [128, total_groups], bf16,
                                     isOutput=False)
    val_p = nc.declare_dram_parameter("val", [128, total_groups], bf16,
                                      isOutput=False)
    y_out = nc.declare_dram_parameter("y", [R, D_OUT], f32, isOutput=True)

    with tile.TileContext(nc) as tc:
        with tc.tile_pool(name="dram", bufs=1, space="DRAM") as dramp, \
             tc.tile_pool(name="const", bufs=1) as constp, \
             tc.tile_pool(name="hbuf", bufs=1) as hbufp:
            R4, BPC, CHK = cfg.R4, cfg.BPC, cfg.CHUNK
            shards = [[dramp.tile([R4, D_H], bf16, name=f"shard{l}_{c}")
                       for c in range(4)] for l in range(3)]
            tables = [[dramp.tile([CHK, D_H], bf16, name=f"table{l}_{c}",
                                  addr_space="Shared") for c in range(4)]
                      for l in range(3)]
            # last SB whose epilogue touches shard chunk c
            s_last = {c: ((c + 1) * BPC + SB_J - 1) // SB_J - 1
                      for c in range(4)}

            def chunk_collective(layer, c):
                nc.gpsimd.collective_compute(
                    "AllGather", ALU.bypass, ins=[shards[layer][c][:]],
                    outs=[tables[layer][c][:]],
                    replica_groups=[list(range(cfg.NC))])

            w_in_t = constp.tile([128, KI, D_H], bf16)
            nc.sync.dma_start(out=w_in_t[:], in_=w_in_p[:])
            w_hid_t = constp.tile([128, D_H], bf16)
            nc.sync.dma_start(out=w_hid_t[:], in_=w_hid_p[:])
            w_out_t = constp.tile([128, D_OUT], bf16)
            nc.sync.dma_start(out=w_out_t[:], in_=w_out_p[:])
            iota_t = constp.tile([128, 128], bf16)
            nc.sync.dma_start(out=iota_t[:], in_=iota_p[:])
            ident_t = constp.tile([128, 128], f32)
            make_identity(nc, ident_t[:])
            h_buf = hbufp.tile([128, NBLK, D_H], f32)

            # ------------- phase 1: shard of table1 = bf16(x @ W_in)
            with tc.tile_pool(name="dense", bufs=3) as densep, \
                 tc.tile_pool(name="dpsum", bufs=4, space="PSUM") as dpsp:
                for t in range(NBLK):
                    xt = densep.tile([128, KI, 128], bf16, tag="xt")
                    nc.sync.dma_start(out=xt[:], in_=xT_in[t])
                    ps = dpsp.tile([128, D_H], f32, space="PSUM", tag="dps")
                    for j in range(KI):
                        nc.tensor.matmul(ps[:], lhsT=xt[:, j, :],
                                         rhs=w_in_t[:, j, :],
                                         start=(j == 0), stop=(j == KI - 1))
                    st = densep.tile([128, D_H], bf16, tag="stage")
                    nc.scalar.activation(st[:], ps[:], AF.Copy)
                    nc.sync.dma_start(
                        out=shards[0][t // BPC][(t % BPC) * 128:
                                                (t % BPC + 1) * 128, :],
                        in_=st[:])
                    if t % BPC == BPC - 1:
                        chunk_collective(0, t // BPC)

            # per-chunk views for batched per-SB stores:
            # chunk c row (t - c*BPC)*128 + p  <- stage[p, t - s*SB_J, :]
            shard_v = [[shards[l][c].rearrange("(b p) n -> p b n",
                                               b=BPC, p=128)
                        for c in range(4)] for l in range(3)]
            y_v = y_out.rearrange("(s j p) n -> s p j n", s=NSB, j=SB_J,
                                  p=128)

            # ------------- phases 2-4: spmm layers
            for layer in range(cfg.LAYERS):
                table = tables[layer]
                d_l = D_H if layer < 2 else D_OUT
                with tc.tile_pool(name=f"gt{layer}", bufs=6) as gtp, \
                     tc.tile_pool(name=f"wk{layer}", bufs=4) as wkp, \
                     tc.tile_pool(name=f"sg{layer}", bufs=6) as sgp, \
                     tc.tile_pool(name=f"str{layer}", bufs=2) as strp, \
                     tc.tile_pool(name=f"ac{layer}", bufs=4,
                                  space="PSUM") as psp, \
                     tc.tile_pool(name=f"tp{layer}", bufs=2,
                                  space="PSUM") as tpsp:
                    for s in range(NSB):
                        sb_gs = sb_sched(s)
                        if not sb_gs:
                            continue
                        c0 = sb_gs[0]["col_off"]
                        ncols = sum(g["num_idxs"] // 16 for g in sb_gs)
                        g0 = sb_gs[0]["grp_off"]
                        ngrp = sum(g["num_idxs"] // 128 for g in sb_gs)
                        idx_t = strp.tile([128, sbc_cap], i16, tag="idx")
                        nc.sync.dma_start(out=idx_t[:, :ncols],
                                          in_=idx_p[:, c0:c0 + ncols])
                        rl_t = strp.tile([128, sbg_cap], bf16, tag="rl")
                        nc.sync.dma_start(out=rl_t[:, :ngrp],
                                          in_=rl_p[:, g0:g0 + ngrp])
                        val_t = strp.tile([128, sbg_cap], bf16, tag="val")
                        nc.sync.dma_start(out=val_t[:, :ngrp],
                                          in_=val_p[:, g0:g0 + ngrp])

                        blocks = [b for b in sb_blocks[s] if b in live_blocks]

                        # all 4 chunk gathers in flight on 4 SWDGE queues
                        gtiles = []
                        for g in sb_gs:
                            ng = g["num_idxs"] // 128
                            gt = gtp.tile([128, g_cap, 128], bf16, tag="g")
                            nc.gpsimd.dma_gather(
                                out_ap=gt[:, :ng, :],
                                in_ap=table[g["q"]][:],
                                idxs_ap=idx_t[:, g["col_off"] - c0:
                                              g["col_off"] - c0
                                              + g["num_idxs"] // 16],
                                num_idxs=g["num_idxs"],
                                num_idxs_reg=g["num_idxs"],
                                elem_size=D_H,
                                single_packet=cfg.SP,
                                queue_num=g["q"] % cfg.NQ)
                            gtiles.append(gt)

                        # sel batches per chunk (DVE, overlaps the gathers)
                        stiles = []
                        for g in sb_gs:
                            ng = g["num_idxs"] // 128
                            gl = g["grp_off"] - g0   # local group offset
                            sel = sgp.tile([128, g_cap, 128], bf16,
                                           tag="sel")
                            nc.vector.tensor_tensor(
                                out=sel[:, :ng, :],
                                in0=iota_t[:].unsqueeze(1)
                                    .broadcast_to([128, ng, 128]),
                                in1=rl_t[:, gl:gl + ng].unsqueeze(2)
                                    .broadcast_to([128, ng, 128]),
                                op=ALU.is_equal)
                            nc.vector.tensor_tensor(
                                out=sel[:, :ng, :],
                                in0=sel[:, :ng, :],
                                in1=val_t[:, gl:gl + ng].unsqueeze(2)
                                    .broadcast_to([128, ng, 128]),
                                op=ALU.mult)
                            stiles.append(sel)

                        # block-major accumulation: one PSUM bank per block
                        acc_tiles = {}
                        for b in blocks:
                            segs = []
                            for gt, sel, g in zip(gtiles, stiles, sb_gs):
                                for bb, gs, ge in g["segments"]:
                                    if bb == b:
                                        segs.append((gt, sel, gs, ge))
                            n_seg_groups = sum(ge - gs
                                               for _, _, gs, ge in segs)
                            acc = psp.tile([128, 128], f32, space="PSUM",
                                           tag="acc")
                            acc_tiles[b] = acc
                            done = 0
                            for gt, sel, gs, ge in segs:
                                for grp in range(gs, ge):
                                    nc.tensor.matmul(
                                        acc[:, :d_l],
                                        lhsT=sel[:, grp, :],
                                        rhs=gt[:, grp, :d_l],
                                        start=(done == 0),
                                        stop=(done == n_seg_groups - 1))
                                    done += 1

                        # epilogue (+ transform feeding next table)
                        if layer < 2:
                            stage = wkp.tile([128, SB_J, D_H], bf16,
                                             tag="tstage")
                            nc.vector.memset(stage[:], 0)
                        else:
                            stage_y = wkp.tile([128, SB_J, D_OUT], f32,
                                               tag="ystage")
                            nc.vector.memset(stage_y[:], 0)
                        for j, b in enumerate(sb_blocks[s]):
                            real = (b * 128) < cfg.N  # any real rows?
                            if b in acc_tiles:
                                acc = acc_tiles[b]
                                if layer == 0:
                                    nc.scalar.activation(h_buf[:, b, :],
                                                         acc[:], AF.Relu)
                                elif layer == 1:
                                    tmp = wkp.tile([128, D_H], f32,
                                                   tag="tmp")
                                    nc.scalar.activation(tmp[:], acc[:],
                                                         AF.Relu)
                                    nc.vector.tensor_tensor(
                                        out=h_buf[:, b, :], in0=tmp[:],
                                        in1=h_buf[:, b, :], op=ALU.add)
                                else:
                                    nc.vector.tensor_copy(
                                        stage_y[:, j, :],
                                        acc[:, :D_OUT])
                            elif real:
                                if layer == 0:
                                    nc.vector.memset(h_buf[:, b, :], 0)
                                elif layer == 2:
                                    pass  # stage_y already zero
                            else:
                                continue  # fully fake block
                            if cfg.XFORM and layer < 2 and (b in acc_tiles or real):
                                w_next = w_hid_t if layer == 0 else w_out_t
                                d_n = D_H if layer == 0 else D_OUT
                                tp = tpsp.tile([128, 128], f32, space="PSUM",
                                               tag="tp")
                                nc.tensor.transpose(tp[:], h_buf[:, b, :],
                                                    ident_t[:])
                                hT = wkp.tile([128, 128], bf16, tag="hT")
                                nc.vector.tensor_copy(hT[:], tp[:])
                                tp2 = tpsp.tile([128, 128], f32,
                                                space="PSUM", tag="tp2")
                                nc.tensor.matmul(tp2[:, :d_n], lhsT=hT[:],
                                                 rhs=w_next[:, :d_n],
                                                 start=True, stop=True)
                                nc.scalar.activation(stage[:, j, :d_n],
                                                     tp2[:, :d_n], AF.Copy)
                        if layer < 2:
                            b0 = s * SB_J
                            j0 = 0
                            while j0 < SB_J:
                                c = (b0 + j0) // BPC
                                j1 = min(SB_J, (c + 1) * BPC - b0)
                                nc.sync.dma_start(
                                    out=shard_v[layer + 1][c][
                                        :, b0 + j0 - c * BPC:
                                        b0 + j1 - c * BPC, :],
                                    in_=stage[:, j0:j1, :])
                                j0 = j1
                            for c in range(4):
                                if s_last[c] == s:
                                    chunk_collective(layer + 1, c)
                        else:
                            nc.sync.dma_start(out=y_v[s], in_=stage_y[:])

    nc.compile()
    return nc


# ------------------------------------------------------------------- driver
def prepare_inputs(x, W_in, W_hid, W_out, new_id, streams, cfg):
    """Build per-core in_maps."""
    KI = cfg.D_IN // 128
    x_pad = np.zeros((cfg.N_PAD, cfg.D_IN), np.float32)
    x_pad[new_id] = np.asarray(x, np.float32)
    # block-major xT: [NBLK, 128 feat-part, KI, 128 node] per core, so each
    # per-block DMA reads 1KB-contiguous per partition
    x_blk = x_pad.reshape(cfg.NC, cfg.NBLK, 128, KI, 128)  # [c,t,n,j,p]
    x_blk = np.ascontiguousarray(
        x_blk.transpose(0, 1, 4, 3, 2)).astype(BF16)       # [c,t,p,j,n]

    w_in_t = np.asarray(W_in, np.float32).reshape(KI, 128, cfg.D_H)
    w_in_t = np.ascontiguousarray(w_in_t.transpose(1, 0, 2)).astype(BF16)
    w_hid_t = np.asarray(W_hid, np.float32).astype(BF16)
    w_out_t = np.asarray(W_out, np.float32).astype(BF16)
    iota = np.tile(np.arange(128, dtype=np.float32), (128, 1)).astype(BF16)

    in_maps = []
    for c in range(cfg.NC):
        st = streams[c]
        in_maps.append({
            "xT": x_blk[c],
            "w_in": w_in_t, "w_hid": w_hid_t, "w_out": w_out_t,
            "iota": iota,
            "idxs": st["idx_stream"].astype(np.int16),
            "rl": st["rl_stream"].astype(BF16),
            "val": st["val_stream"].astype(BF16),
        })
    return in_maps


def assemble_output(results, new_id, cfg):
    y_pad = np.concatenate([results[c]["y"] for c in range(cfg.NC)], axis=0)
    return np.ascontiguousarray(y_pad[new_id]).astype(np.float32)


_CACHE = {}


def run(x, adj_rows, adj_cols, adj_vals, W_in, W_hid, W_out,
        cfg=DEFAULT_CFG, trace=False):
    from concourse.bass_utils import run_bass_kernel_spmd
    adj_rows = np.asarray(adj_rows)
    adj_cols = np.asarray(adj_cols)
    adj_vals = np.asarray(adj_vals, np.float32)
    key = ("plan", adj_rows.tobytes()[:64], cfg.N, cfg.E, cfg.NBLK)
    if key not in _CACHE:
        new_id, meta, streams = build_plan(adj_rows, adj_cols, adj_vals, cfg)
        nc = build_program(meta, cfg)
        _CACHE[key] = (new_id, meta, streams, nc)
    new_id, meta, streams, nc = _CACHE[key]
    in_maps = prepare_inputs(x, W_in, W_hid, W_out, new_id, streams, cfg)
    kw = {}
    if trace:
        try:
            import ntff_hook
            ntff_hook.install()
            kw["trace"] = True
        except Exception:
            pass
    res = run_bass_kernel_spmd(nc, in_maps, list(range(cfg.NC)), **kw)
    out = assemble_output(res.results, new_id, cfg)
    return out, res


def kernel(x, adj_rows, adj_cols, adj_vals, W_in, W_hid, W_out):
    out, _ = run(x, adj_rows, adj_cols, adj_vals, W_in, W_hid, W_out)
    return out



# revision 39
# speedup vs baseline: 2.8516x; 1.0376x over previous
"""GCN (3-layer graph conv) Trainium2 kernel running SPMD on 8 NeuronCores.

Approach
--------
- Destination-node 1D sharding; nodes renumbered (degree-balanced snake
  deal across cores, round-robin deal across blocks within a core) and
  padded so each core owns R rows = NBLK blocks of 128.
- Per layer a bf16 feature table (4 chunk tensors of NC*R/4 rows in HBM,
  replicated per core via per-chunk AllGathers issued as soon as each
  chunk's shard rows are produced, overlapping collectives with compute)
  is randomly gathered with gpsimd.dma_gather (int16 indices; each chunk
  pinned to its own SWDGE queue 0-3 so all four Q7 core pairs generate
  descriptors concurrently).
- SpMM: for each 128-row destination block, one-hot selection matrices
  (built in multi-group batches with two DVE tensor_tensor ops over
  stride-0 broadcast APs: is_equal(iota, rl) then * val) x gathered
  tiles (PE matmul) accumulate the block result in PSUM.
- The schedule is UNIFORM across cores (group counts per (block, chunk)
  padded to the max over cores) so one SPMD program serves all 8 cores;
  per-core behavior differs only through the idx/row-local/val streams.

kernel(**inputs) accepts the full-size inputs from reference.setup_inputs
and returns the full [100000, 64] float32 output.
"""
import sys

sys.path.insert(0, "/opt/trn_rl_repo")

import numpy as np
import ml_dtypes

BF16 = ml_dtypes.bfloat16


class Cfg:
    def __init__(self, n=100000, e=1600000, nsb=13, sb_j=8,
                 d_in=512, d_h=128, d_out=64, gather_single_packet=False,
                 layers=3, xform=True, n_queues=4):
        self.N = n
        self.E = e
        self.NC = 8
        self.NSB = nsb              # super-blocks per core
        self.SB_J = sb_j            # blocks per super-block
        self.NBLK = nsb * sb_j      # blocks per core
        self.R = self.NBLK * 128    # rows per core
        self.N_PAD = self.NC * self.R
        assert self.R % (4 * 128) == 0
        self.R4 = self.R // 4       # shard rows per table chunk
        self.BPC = self.R4 // 128   # blocks per chunk
        self.CHUNK = self.NC * self.R4   # rows per chunk table
        assert self.CHUNK <= 32767, "int16 gather index range"
        assert self.R >= (n + self.NC - 1) // self.NC
        self.D_IN = d_in
        self.D_H = d_h
        self.D_OUT = d_out
        self.SP = gather_single_packet
        self.LAYERS = layers
        self.XFORM = xform
        self.NQ = n_queues


DEFAULT_CFG = Cfg()


# ------------------------------------------------------------------ planning
def _node_assignment(adj_rows, cfg):
    """new_id[orig] -> padded id.  Degree-balanced + degree-sorted."""
    deg = np.bincount(adj_rows, minlength=cfg.N)
    order = np.argsort(-deg, kind="stable")
    snake = np.concatenate([np.arange(cfg.NC), np.arange(cfg.NC)[::-1]])
    cores_for_pos = snake[np.arange(cfg.N) % (2 * cfg.NC)]
    new_id = np.empty(cfg.N, dtype=np.int64)
    for c in range(cfg.NC):
        nodes = order[cores_for_pos == c]
        r = np.arange(len(nodes))
        # deal degree-ranked nodes round-robin across blocks: every block
        # gets a balanced degree mix (SBs and table chunks stay uniform)
        pos = (r % cfg.NBLK) * 128 + r // cfg.NBLK
        new_id[nodes] = c * cfg.R + pos
    return new_id


def build_plan(adj_rows, adj_cols, adj_vals, cfg):
    """Returns (new_id, G_u, schedule, per-core streams).

    G_u[b, q]: uniform group count per (block, chunk).
    schedule: list over (sb, q) in processing order of dicts
      {sb, q, num_idxs, col_off, grp_off, segments=[(b, gs, ge), ...]}
    streams[c]: dict(idx_stream [128, cols] i16, rl_stream [128, G] f32,
                     val_stream [128, G] f32)
    """
    new_id = _node_assignment(adj_rows, cfg)
    dest = new_id[adj_rows]
    src = new_id[adj_cols]
    core = (dest // cfg.R).astype(np.int64)
    local = dest % cfg.R
    block = (local // 128).astype(np.int64)
    row_local = (local % 128).astype(np.float32)
    # table chunk c holds every core's shard rows [c*R4, (c+1)*R4)
    src_core = src // cfg.R
    src_loc = src % cfg.R
    chunk = (src_loc // cfg.R4).astype(np.int64)
    idx16 = (src_core * cfg.R4 + src_loc % cfg.R4).astype(np.int16)

    counts = np.zeros((cfg.NC, cfg.NBLK, 4), dtype=np.int64)
    np.add.at(counts, (core, block, chunk), 1)
    G_u = np.ceil(counts / 128).astype(np.int64).max(axis=0)  # [NBLK, 4]
    mean_cnt = counts.mean(axis=0)  # [NBLK, 4]

    # schedule (same for all cores); most-padded block last per (s, q) so
    # its pad slots are trailing in the gather stream (idx=-1 -> dropped)
    sb_blocks = {s: [s * cfg.SB_J + j for j in range(cfg.SB_J)]
                 for s in range(cfg.NSB)}
    schedule = []
    col_off = 0
    grp_off = 0
    for s in range(cfg.NSB):
        for q in range(4):
            order = sorted(sb_blocks[s],
                           key=lambda b: G_u[b, q] * 128 - mean_cnt[b, q])
            segments = []
            cur = 0
            for b in order:
                g = int(G_u[b, q])
                if g:
                    segments.append((b, cur, cur + g))
                    cur += g
            if cur == 0:
                continue
            num = cur * 128
            schedule.append(dict(sb=s, q=q, num_idxs=num, col_off=col_off,
                                 grp_off=grp_off, segments=segments))
            col_off += num // 16
            grp_off += cur
    total_cols = col_off
    total_groups = grp_off

    # per-core streams
    sb_of_block = np.arange(cfg.NBLK) // cfg.SB_J
    streams = []
    for c in range(cfg.NC):
        m = core == c
        b_e = block[m]; rl_e = row_local[m]; q_e = chunk[m]
        ix_e = idx16[m]; v_e = adj_vals[m].astype(np.float32)
        order_e = np.lexsort((rl_e, b_e, q_e, sb_of_block[b_e]))
        b_e = b_e[order_e]; rl_e = rl_e[order_e]; q_e = q_e[order_e]
        ix_e = ix_e[order_e]; v_e = v_e[order_e]

        ix_slots = np.zeros(total_groups * 128, np.int16)
        rl_slots = np.full(total_groups * 128, -1.0, np.float32)
        v_slots = np.zeros(total_groups * 128, np.float32)
        # each (sb, q, block) run lands at its schedule slot offset
        keys = (sb_of_block[b_e] * 8 + q_e) * cfg.NBLK + b_e
        uniq, starts, cnts = np.unique(keys, return_index=True,
                                       return_counts=True)
        run_of_key = {}
        for g in schedule:
            for b, gs, ge in g["segments"]:
                k = (g["sb"] * 8 + g["q"]) * cfg.NBLK + b
                run_of_key[k] = (g["grp_off"] + gs) * 128
        for k, st, cn in zip(uniq, starts, cnts):
            slot0 = run_of_key[int(k)]
            sl = slice(slot0, slot0 + cn)
            ix_slots[sl] = ix_e[st:st + cn]
            rl_slots[sl] = rl_e[st:st + cn]
            v_slots[sl] = v_e[st:st + cn]

        # NOTE: trailing idx=-1 dropping requires num_idxs_reg to carry the
        # per-core post-drop count (ring bookkeeping at decode advances by
        # the register count; a mismatch desyncs the descriptor ring and
        # hangs the device). Pads keep idx 0 until that is wired up.
        call_counts = []
        for g in schedule:
            a = g["grp_off"] * 128
            e_ = a + g["num_idxs"]
            real = np.nonzero(rl_slots[a:e_] >= 0)[0]
            call_counts.append(int(real[-1]) + 1 if len(real) else 0)
        call_counts = np.asarray(call_counts, np.int32)

        # idx layout per gather: [128, num/16] idx j -> [j%16, j//16], x8
        idx_cols = np.empty((128, total_cols), np.int16)
        for g in schedule:
            n = g["num_idxs"]
            seg = ix_slots[g["grp_off"] * 128: g["grp_off"] * 128 + n]
            tile16 = seg.reshape(n // 16, 16).T          # [16, n/16]
            idx_cols[:, g["col_off"]: g["col_off"] + n // 16] = np.tile(
                tile16, (8, 1))
        rl_stream = rl_slots.reshape(total_groups, 128).T
        val_stream = v_slots.reshape(total_groups, 128).T
        streams.append(dict(
            idx_stream=np.ascontiguousarray(idx_cols),
            rl_stream=np.ascontiguousarray(rl_stream),
            val_stream=np.ascontiguousarray(val_stream),
            call_counts=call_counts))

    meta = dict(total_cols=total_cols, total_groups=total_groups,
                G_u=G_u, schedule=schedule, sb_blocks=sb_blocks)
    return new_id, meta, streams


# ------------------------------------------------------------ device program
def build_program(meta, cfg):
    from concourse import bacc, mybir, tile
    from concourse.masks import make_identity

    f32 = mybir.dt.float32
    bf16 = mybir.dt.bfloat16
    i16 = mybir.dt.int16
    AF = mybir.ActivationFunctionType
    ALU = mybir.AluOpType

    schedule = meta["schedule"]
    sb_blocks = meta["sb_blocks"]
    total_cols = meta["total_cols"]
    total_groups = meta["total_groups"]
    NSB, SB_J, NBLK = cfg.NSB, cfg.SB_J, cfg.NBLK
    R, N_PAD, CHUNK = cfg.R, cfg.N_PAD, cfg.CHUNK
    D_IN, D_H, D_OUT = cfg.D_IN, cfg.D_H, cfg.D_OUT
    KI = D_IN // 128

    g_cap = max(g["num_idxs"] // 128 for g in schedule)
    def sb_sched(s):
        return [g for g in schedule if g["sb"] == s]
    sbc_cap = max(sum(g["num_idxs"] // 16 for g in sb_sched(s))
                  for s in range(NSB))
    sbg_cap = max(sum(g["num_idxs"] // 128 for g in sb_sched(s))
                  for s in range(NSB))
    # blocks with any edges (uniform over cores)
    live_blocks = {b for g in schedule for (b, _, _) in g["segments"]}

    nc = bacc.Bacc("TRN2", target_bir_lowering=False,
                   num_swdge_queues=cfg.NQ)

    xT_in = nc.declare_dram_parameter("xT", [NBLK, 128, KI, 128], bf16,
                                      isOutput=False)
    w_in_p = nc.declare_dram_parameter("w_in", [128, KI, D_H], bf16,
                                       isOutput=False)
    w_hid_p = nc.declare_dram_parameter("w_hid", [128, D_H], bf16,
                                        isOutput=False)
    w_out_p = nc.declare_dram_parameter("w_out", [128, D_OUT], bf16,
                                        isOutput=False)
    iota_p = nc.declare_dram_parameter("iota", [128, 128], bf16,
                                       isOutput=False)
    idx_p = nc.declare_dram_parameter("idxs", [128, total_cols], i16,
                                      isOutput=False)
    rl_p = nc.declare_dram_parameter("rl", [128, total_groups], bf16,
                                     isOutput=False)
    val_p = nc.declare_dram_parameter("val", [128, total_groups], bf16,
                                      isOutput=False)
    y_out = nc.declare_dram_parameter("y", [R, D_OUT], f32, isOutput=True)

    with tile.TileContext(nc) as tc:
        with tc.tile_pool(name="dram", bufs=1, space="DRAM") as dramp, \
             tc.tile_pool(name="const", bufs=1) as constp, \
             tc.tile_pool(name="hbuf", bufs=1) as hbufp:
            R4, BPC, CHK = cfg.R4, cfg.BPC, cfg.CHUNK
            shards = [[dramp.tile([R4, D_H], bf16, name=f"shard{l}_{c}")
                       for c in range(4)] for l in range(3)]
            tables = [[dramp.tile([CHK, D_H], bf16, name=f"table{l}_{c}",
                                  addr_space="Shared") for c in range(4)]
                      for l in range(3)]
            # last SB whose epilogue touches shard chunk c
            s_last = {c: ((c + 1) * BPC + SB_J - 1) // SB_J - 1
                      for c in range(4)}

            def chunk_collective(layer, c):
                nc.gpsimd.collective_compute(
                    "AllGather", ALU.bypass, ins=[shards[layer][c][:]],
                    outs=[tables[layer][c][:]],
                    replica_groups=[list(range(cfg.NC))])

            w_in_t = constp.tile([128, KI, D_H], bf16)
            nc.sync.dma_start(out=w_in_t[:], in_=w_in_p[:])
            w_hid_t = constp.tile([128, D_H], bf16)
            nc.sync.dma_start(out=w_hid_t[:], in_=w_hid_p[:])
            w_out_t = constp.tile([128, D_OUT], bf16)
            nc.sync.dma_start(out=w_out_t[:], in_=w_out_p[:])
            iota_t = constp.tile([128, 128], bf16)
            nc.sync.dma_start(out=iota_t[:], in_=iota_p[:])
            ident_t = constp.tile([128, 128], f32)
            make_identity(nc, ident_t[:])
            h_buf = hbufp.tile([128, NBLK, D_H], f32)

            # ------------- phase 1: shard of table1 = bf16(x @ W_in)
            with tc.tile_pool(name="dense", bufs=3) as densep, \
                 tc.tile_pool(name="dpsum", bufs=4, space="PSUM") as dpsp:
                for t in range(NBLK):
                    xt = densep.tile([128, KI, 128], bf16, tag="xt")
                    nc.sync.dma_start(out=xt[:], in_=xT_in[t])
                    ps = dpsp.tile([128, D_H], f32, space="PSUM", tag="dps")
                    for j in range(KI):
                        nc.tensor.matmul(ps[:], lhsT=xt[:, j, :],
                                         rhs=w_in_t[:, j, :],
                                         start=(j == 0), stop=(j == KI - 1))
                    st = densep.tile([128, D_H], bf16, tag="stage")
                    nc.scalar.activation(st[:], ps[:], AF.Copy)
                    nc.sync.dma_start(
                        out=shards[0][t // BPC][(t % BPC) * 128:
                                                (t % BPC + 1) * 128, :],
                        in_=st[:])
                    if t % BPC == BPC - 1:
                        chunk_collective(0, t // BPC)

            # per-chunk views for batched per-SB stores:
            # chunk c row (t - c*BPC)*128 + p  <- stage[p, t - s*SB_J, :]
            shard_v = [[shards[l][c].rearrange("(b p) n -> p b n",
                                               b=BPC, p=128)
                        for c in range(4)] for l in range(3)]
            y_v = y_out.rearrange("(s j p) n -> s p j n", s=NSB, j=SB_J,
                                  p=128)

            # ------------- phases 2-4: spmm layers
            for layer in range(cfg.LAYERS):
                table = tables[layer]
                d_l = D_H if layer < 2 else D_OUT
                with tc.tile_pool(name=f"gt{layer}", bufs=7) as gtp, \
                     tc.tile_pool(name=f"wk{layer}", bufs=4) as wkp, \
                     tc.tile_pool(name=f"sg{layer}", bufs=6) as sgp, \
                     tc.tile_pool(name=f"str{layer}", bufs=3) as strp, \
                     tc.tile_pool(name=f"ac{layer}", bufs=6,
                                  space="PSUM") as psp, \
                     tc.tile_pool(name=f"tp{layer}", bufs=1,
                                  space="PSUM") as tpsp:
                    for s in range(NSB):
                        sb_gs = sb_sched(s)
                        if not sb_gs:
                            continue
                        c0 = sb_gs[0]["col_off"]
                        ncols = sum(g["num_idxs"] // 16 for g in sb_gs)
                        g0 = sb_gs[0]["grp_off"]
                        ngrp = sum(g["num_idxs"] // 128 for g in sb_gs)
                        idx_t = strp.tile([128, sbc_cap], i16, tag="idx")
                        nc.sync.dma_start(out=idx_t[:, :ncols],
                                          in_=idx_p[:, c0:c0 + ncols])
                        rl_t = strp.tile([128, sbg_cap], bf16, tag="rl")
                        nc.sync.dma_start(out=rl_t[:, :ngrp],
                                          in_=rl_p[:, g0:g0 + ngrp])
                        val_t = strp.tile([128, sbg_cap], bf16, tag="val")
                        nc.sync.dma_start(out=val_t[:, :ngrp],
                                          in_=val_p[:, g0:g0 + ngrp])

                        blocks = [b for b in sb_blocks[s] if b in live_blocks]

                        # all 4 chunk gathers in flight on 4 SWDGE queues
                        gtiles = []
                        for g in sb_gs:
                            ng = g["num_idxs"] // 128
                            gt = gtp.tile([128, g_cap, 128], bf16, tag="g")
                            nc.gpsimd.dma_gather(
                                out_ap=gt[:, :ng, :],
                                in_ap=table[g["q"]][:],
                                idxs_ap=idx_t[:, g["col_off"] - c0:
                                              g["col_off"] - c0
                                              + g["num_idxs"] // 16],
                                num_idxs=g["num_idxs"],
                                num_idxs_reg=g["num_idxs"],
                                elem_size=D_H,
                                single_packet=cfg.SP,
                                queue_num=g["q"] % cfg.NQ)
                            gtiles.append(gt)

                        # sel batches per chunk (DVE, overlaps the gathers)
                        stiles = []
                        for g in sb_gs:
                            ng = g["num_idxs"] // 128
                            gl = g["grp_off"] - g0   # local group offset
                            sel = sgp.tile([128, g_cap, 128], bf16,
                                           tag="sel")
                            nc.vector.tensor_tensor(
                                out=sel[:, :ng, :],
                                in0=iota_t[:].unsqueeze(1)
                                    .broadcast_to([128, ng, 128]),
                                in1=rl_t[:, gl:gl + ng].unsqueeze(2)
                                    .broadcast_to([128, ng, 128]),
                                op=ALU.is_equal)
                            nc.vector.tensor_tensor(
                                out=sel[:, :ng, :],
                                in0=sel[:, :ng, :],
                                in1=val_t[:, gl:gl + ng].unsqueeze(2)
                                    .broadcast_to([128, ng, 128]),
                                op=ALU.mult)
                            stiles.append(sel)

                        # block-major accumulation: one PSUM bank per block
                        acc_tiles = {}
                        for b in blocks:
                            segs = []
                            for gt, sel, g in zip(gtiles, stiles, sb_gs):
                                for bb, gs, ge in g["segments"]:
                                    if bb == b:
                                        segs.append((gt, sel, gs, ge))
                            n_seg_groups = sum(ge - gs
                                               for _, _, gs, ge in segs)
                            acc = psp.tile([128, 128], f32, space="PSUM",
                                           tag="acc")
                            acc_tiles[b] = acc
                            done = 0
                            for gt, sel, gs, ge in segs:
                                for grp in range(gs, ge):
                                    nc.tensor.matmul(
                                        acc[:, :d_l],
                                        lhsT=sel[:, grp, :],
                                        rhs=gt[:, grp, :d_l],
                                        start=(done == 0),
                                        stop=(done == n_seg_groups - 1))
                                    done += 1

                        # epilogue (+ transform feeding next table)
                        if layer < 2:
                            stage = wkp.tile([128, SB_J, D_H], bf16,
                                             tag="tstage")
                            nc.vector.memset(stage[:], 0)
                        else:
                            stage_y = wkp.tile([128, SB_J, D_OUT], f32,
                                               tag="ystage")
                            nc.vector.memset(stage_y[:], 0)
                        for j, b in enumerate(sb_blocks[s]):
                            real = (b * 128) < cfg.N  # any real rows?
                            if b in acc_tiles:
                                acc = acc_tiles[b]
                                if layer == 0:
                                    nc.scalar.activation(h_buf[:, b, :],
                                                         acc[:], AF.Relu)
                                elif layer == 1:
                                    tmp = wkp.tile([128, D_H], f32,
                                                   tag="tmp")
                                    nc.scalar.activation(tmp[:], acc[:],
                                                         AF.Relu)
                                    nc.vector.tensor_tensor(
                                        out=h_buf[:, b, :], in0=tmp[:],
                                        in1=h_buf[:, b, :], op=ALU.add)
                                else:
                                    nc.vector.tensor_copy(
                                        stage_y[:, j, :],
                                        acc[:, :D_OUT])
                            elif real:
                                if layer == 0:
                                    nc.vector.memset(h_buf[:, b, :], 0)
                                elif layer == 2:
                                    pass  # stage_y already zero
                            else:
                                continue  # fully fake block
                            if cfg.XFORM and layer < 2 and (b in acc_tiles or real):
                                w_next = w_hid_t if layer == 0 else w_out_t
                                d_n = D_H if layer == 0 else D_OUT
                                tp = tpsp.tile([128, 128], f32, space="PSUM",
                                               tag="tp")
                                nc.tensor.transpose(tp[:], h_buf[:, b, :],
                                                    ident_t[:])
                                hT = wkp.tile([128, 128], bf16, tag="hT")
                                nc.vector.tensor_copy(hT[:], tp[:])
                                tp2 = tpsp.tile([128, 128], f32,
                                                space="PSUM", tag="tp2")
                                nc.tensor.matmul(tp2[:, :d_n], lhsT=hT[:],
                                                 rhs=w_next[:, :d_n],
                                                 start=True, stop=True)
                                nc.scalar.activation(stage[:, j, :d_n],
                                                     tp2[:, :d_n], AF.Copy)
                        if layer < 2:
                            b0 = s * SB_J
                            j0 = 0
                            while j0 < SB_J:
                                c = (b0 + j0) // BPC
                                j1 = min(SB_J, (c + 1) * BPC - b0)
                                nc.sync.dma_start(
                                    out=shard_v[layer + 1][c][
                                        :, b0 + j0 - c * BPC:
                                        b0 + j1 - c * BPC, :],
                                    in_=stage[:, j0:j1, :])
                                j0 = j1
                            for c in range(4):
                                if s_last[c] == s:
                                    chunk_collective(layer + 1, c)
                        else:
                            nc.sync.dma_start(out=y_v[s], in_=stage_y[:])

    nc.compile()
    return nc


# ------------------------------------------------------------------- driver
def prepare_inputs(x, W_in, W_hid, W_out, new_id, streams, cfg):
    """Build per-core in_maps."""
    KI = cfg.D_IN // 128
    x_pad = np.zeros((cfg.N_PAD, cfg.D_IN), np.float32)
    x_pad[new_id] = np.asarray(x, np.float32)
    # block-major xT: [NBLK, 128 feat-part, KI, 128 node] per core, so each
    # per-block DMA reads 1KB-contiguous per partition
    x_blk = x_pad.reshape(cfg.NC, cfg.NBLK, 128, KI, 128)  # [c,t,n,j,p]
    x_blk = np.ascontiguousarray(
        x_blk.transpose(0, 1, 4, 3, 2)).astype(BF16)       # [c,t,p,j,n]

    w_in_t = np.asarray(W_in, np.float32).reshape(KI, 128, cfg.D_H)
    w_in_t = np.ascontiguousarray(w_in_t.transpose(1, 0, 2)).astype(BF16)
    w_hid_t = np.asarray(W_hid, np.float32).astype(BF16)
    w_out_t = np.asarray(W_out, np.float32).astype(BF16)
    iota = np.tile(np.arange(128, dtype=np.float32), (128, 1)).astype(BF16)

    in_maps = []
    for c in range(cfg.NC):
        st = streams[c]
        in_maps.append({
            "xT": x_blk[c],
            "w_in": w_in_t, "w_hid": w_hid_t, "w_out": w_out_t,
            "iota": iota,
            "idxs": st["idx_stream"].astype(np.int16),
            "rl": st["rl_stream"].astype(BF16),
            "val": st["val_stream"].astype(BF16),
        })
    return in_maps


def assemble_output(results, new_id, cfg):
    y_pad = np.concatenate([results[c]["y"] for c in range(cfg.NC)], axis=0)
    return np.ascontiguousarray(y_pad[new_id]).astype(np.float32)


_CACHE = {}


def run(x, adj_rows, adj_cols, adj_vals, W_in, W_hid, W_out,
        cfg=DEFAULT_CFG, trace=False):
    from concourse.bass_utils import run_bass_kernel_spmd
    adj_rows = np.asarray(adj_rows)
    adj_cols = np.asarray(adj_cols)
    adj_vals = np.asarray(adj_vals, np.float32)
    key = ("plan", adj_rows.tobytes()[:64], cfg.N, cfg.E, cfg.NBLK)
    if key not in _CACHE:
        new_id, meta, streams = build_plan(adj_rows, adj_cols, adj_vals, cfg)
        nc = build_program(meta, cfg)
        _CACHE[key] = (new_id, meta, streams, nc)
    new_id, meta, streams, nc = _CACHE[key]
    in_maps = prepare_inputs(x, W_in, W_hid, W_out, new_id, streams, cfg)
    kw = {}
    if trace:
        try:
            import ntff_hook
            ntff_hook.install()
            kw["trace"] = True
        except Exception:
            pass
    res = run_bass_kernel_spmd(nc, in_maps, list(range(cfg.NC)), **kw)
    out = assemble_output(res.results, new_id, cfg)
    return out, res


def kernel(x, adj_rows, adj_cols, adj_vals, W_in, W_hid, W_out):
    out, _ = run(x, adj_rows, adj_cols, adj_vals, W_in, W_hid, W_out)
    return out

